# revision 1
# baseline (speedup 1.0000x reference)
"""EMA-decomposition kernel for Trainium2 (8 NeuronCores, Bass/Tile).

Problem: x [32, 4096, 512] f32; EMA along time (alpha=0.3):
    s_0 = x_0, s_t = a*x_t + (1-a)*s_{t-1}
Returns (x - s, s).

Key math: with a=0.3, the per-128-step block decay (0.7)^128 ~ 1.5e-20 is
far below fp32 resolution, so the scan carry beyond one 128-step block is
numerically zero.  Each 128-row output block is exactly (to fp32):
    s_blk[j] = M  @ x_blk[j]   + D @ x_blk[j-1]      (j >= 1)
    s_blk[0] = M0 @ x_blk[0]
with constant 128x128 matrices:
    M[t,k]  = a*(1-a)^(t-k)  for k<=t else 0
    M0      = M with column 0 replaced by (1-a)^t   (s_0 = x_0 boundary)
    D[t,k]  = a*(1-a)^(t+128-k)
So the whole scan becomes independent TensorE matmuls (no sequential
dependency at all).

Sharding: batch dim 32 -> 4 per core (embarrassingly parallel; time axis
never sharded).  Per core traffic: 32 MiB in + 64 MiB out -> ~280 us DMA
roofline per core at ~358 GB/s.

MODE:
  "f32"   — exact fp32 matmuls (4 cyc/row, PE ~305 us busy)
  "split" — split-precision float32r (tf32-rate, 1 cyc/row): x = xr + xl,
            W = Wr + Wl (rounded on device so rounding matches HW exactly),
            W@x ~= Wr@xr + Wr@xl + Wl@xr (dropping Wl@xl ~ 2^-22).
            PE ~162 us busy, error ~1e-6.
  "tf32"  — single rounded f32r matmuls (fastest PE, error ~1e-3)
"""

import numpy as np

import concourse.bass as bass
import concourse.mybir as mybir
from concourse import bass_utils
from concourse.tile import TileContext

ALPHA = 0.3
B, L, C = 32, 4096, 512
N_CORES = 8
B_LOC = B // N_CORES          # 4 sequences per core
P = 128                       # partition dim == time-block size
N_BLK = L // P                # 32 blocks per sequence
MEGA = 8                      # blocks per megatile (DMA granularity: 2 MiB)
N_MEGA = N_BLK // MEGA        # 4 megatiles per sequence

MODE = "split"


def _build_weights():
    """lhsT layouts ([k, t] so that out = lhsT.T @ rhs)."""
    a = float(ALPHA)
    q = 1.0 - a
    k = np.arange(P, dtype=np.float64)[:, None]
    t = np.arange(P, dtype=np.float64)[None, :]
    e = t - k
    with np.errstate(under="ignore"):
        lhsT_m = np.where(e >= 0, a * q ** np.maximum(e, 0.0), 0.0)
        lhsT_m0 = lhsT_m.copy()
        lhsT_m0[0, :] = q ** t[0]
        lhsT_d = a * q ** (e + P)
    return (
        lhsT_m.astype(np.float32),
        lhsT_m0.astype(np.float32),
        lhsT_d.astype(np.float32),
    )


def _build_bass(repeat: int = 1, mode: str | None = None) -> bass.Bass:
    """repeat>1 wraps the whole body in a hardware For_i loop — used only for
    benchmarking (amortizes the ~100ms axon dispatch floor)."""
    mode = MODE if mode is None else mode
    nc = bass.Bass(trn_type="TRN2")
    f32 = mybir.dt.float32
    f32r = mybir.dt.float32r

    x_d = nc.dram_tensor("x", [B_LOC, L, C], f32, kind="ExternalInput")
    wm_d = nc.dram_tensor("wm", [P, P], f32, kind="ExternalInput")
    wm0_d = nc.dram_tensor("wm0", [P, P], f32, kind="ExternalInput")
    wd_d = nc.dram_tensor("wd", [P, P], f32, kind="ExternalInput")
    res_d = nc.dram_tensor("res", [B_LOC, L, C], f32, kind="ExternalOutput")
    ma_d = nc.dram_tensor("ma", [B_LOC, L, C], f32, kind="ExternalOutput")

    with TileContext(nc) as tc:
        with (
            tc.tile_pool(name="wpool", bufs=1) as wpool,
            tc.tile_pool(name="xpool", bufs=9) as xpool,
            tc.tile_pool(name="xrpool", bufs=4) as xrpool,
            tc.tile_pool(name="xlpool", bufs=4) as xlpool,
            tc.tile_pool(name="mapool", bufs=2) as mapool,
            tc.tile_pool(name="pspool", bufs=8, space="PSUM") as pspool,
        ):
            # ---- weights ----
            # Weight DMAs ride ACT's HWDGE queue so SP can start streaming
            # x immediately (weights are off the DMA critical path).
            w32 = {}
            for name, dram in (("m", wm_d), ("m0", wm0_d), ("d", wd_d)):
                t = wpool.tile([P, P], f32, name=f"w32_{name}")
                nc.scalar.dma_start(out=t, in_=dram[:, :])
                w32[name] = t

            if mode == "f32":
                wmm = {name: [t] for name, t in w32.items()}
            else:
                wmm = {}
                for name, t in w32.items():
                    wr = wpool.tile([P, P], f32r, name=f"wr_{name}")
                    nc.vector.tensor_copy(out=wr, in_=t)
                    if mode == "split":
                        wl = wpool.tile([P, P], f32r, name=f"wl_{name}")
                        nc.vector.tensor_sub(out=wl, in0=t, in1=wr.bitcast(f32))
                        wmm[name] = [wr, wl]
                    else:
                        wmm[name] = [wr]

            def mm_terms(wname, cur_parts):
                """(lhsT, rhs) accumulation terms for W @ x."""
                ws = wmm[wname]
                if mode == "split":
                    xr, xl = cur_parts
                    return [(ws[0], xr), (ws[0], xl), (ws[1], xr)]
                return [(ws[0], cur_parts[0])]

            def body():
                for b in range(B_LOC):
                    # [N_MEGA, P, MEGA, C] view of this sequence
                    xr_ = x_d[b].rearrange("(g j p) c -> g p j c", j=MEGA, p=P)
                    mar = ma_d[b].rearrange("(g j p) c -> g p j c", j=MEGA, p=P)
                    resr = res_d[b].rearrange("(g j p) c -> g p j c", j=MEGA, p=P)
                    # Emit ALL input DMAs for this sequence first: SP's queue
                    # is then pure prefetch (stalls only on xt slot recycle),
                    # never behind output waits.
                    xts = []
                    for g in range(N_MEGA):
                        xt = xpool.tile([P, MEGA, C], f32, name="xt")
                        nc.sync.dma_start(out=xt, in_=xr_[g])
                        xts.append(xt)
                    prev_parts = None
                    for g in range(N_MEGA):
                        xt = xts[g]
                        mat = mapool.tile([P, MEGA, C], f32, name="mat")
                        for j in range(MEGA):
                            ps = pspool.tile([P, C], f32, name="ps")
                            if mode == "f32":
                                cur = [xt[:, j, :]]
                            else:
                                # per-block rounding into small ring tiles
                                xrb = xrpool.tile([P, C], f32r, name="xrb")
                                nc.vector.tensor_copy(out=xrb, in_=xt[:, j, :])
                                if mode == "split":
                                    xlb = xlpool.tile([P, C], f32r, name="xlb")
                                    nc.vector.tensor_sub(
                                        out=xlb, in0=xt[:, j, :],
                                        in1=xrb.bitcast(f32),
                                    )
                                    cur = [xrb, xlb]
                                else:
                                    cur = [xrb]
                            terms = []
                            if g == 0 and j == 0:
                                terms += mm_terms("m0", cur)
                            else:
                                terms += mm_terms("m", cur)
                                terms += mm_terms("d", prev_parts)
                            n = len(terms)
                            for i, (lhsT, rhs) in enumerate(terms):
                                nc.tensor.matmul(
                                    ps, lhsT, rhs,
                                    start=(i == 0), stop=(i == n - 1),
                                )
                            # Single PSUM consumer (ACT).
                            nc.scalar.copy(out=mat[:, j, :], in_=ps)
                            prev_parts = cur

                        # res = x - ma fused over the whole megatile, IN
                        # PLACE into the x tile (frees a whole pool; the xt
                        # slot then recycles on res-DMA completion).
                        nc.vector.tensor_sub(out=xt, in0=xt, in1=mat)
                        # ma out via ACT's HWDGE queue (follows its own psum
                        # copies in-order: no wait); res out via the idle
                        # GpSimd SWDGE queue so neither SP (input prefetch)
                        # nor ACT ever stalls on a data wait.  SWDGE DMAs
                        # break walrus codegen inside a For_i, so the bench
                        # variant (repeat>1) falls back to SP for res.
                        nc.scalar.dma_start(out=mar[g], in_=mat)
                        res_q = nc.gpsimd if repeat == 1 else nc.sync
                        res_q.dma_start(out=resr[g], in_=xt)

            if repeat > 1:
                with tc.For_i(0, repeat, 1):
                    body()
            else:
                body()
    return nc


def _split_multi_waits(nc: bass.Bass) -> None:
    """Walrus codegen in this container allows only ONE semaphore wait per
    instruction ("Too many sync wait commands").  Tile's sem assigner emits
    several.  Split: hoist all but one wait onto same-engine NoOps placed
    immediately before the instruction (engines execute their stream in
    order, so this is semantically identical)."""
    n_nops = 0
    for fn in nc.m.functions:
        for blk in fn.blocks:
            out = []
            for inst in blk.instructions:
                si = inst.sync_info
                if si is not None and si.on_wait and len(si.on_wait) > 1:
                    waits = list(si.on_wait)
                    for w in waits[:-1]:
                        nop = mybir.InstNoOp(
                            name=f"{inst.name}-wsplit{n_nops}",
                            engine=inst.engine,
                            ins=[],
                            outs=[],
                        )
                        nop.sync_info = mybir.SyncInfo(on_wait=[w], on_update=[])
                        out.append(nop)
                        n_nops += 1
                    si.on_wait = [waits[-1]]
                out.append(inst)
            blk.instructions = out


def _run(x: np.ndarray, trace: bool = False):
    x = np.ascontiguousarray(np.asarray(x, dtype=np.float32))
    assert x.shape == (B, L, C), x.shape
    wm, wm0, wd = _build_weights()
    nc = _build_bass()
    _split_multi_waits(nc)
    in_maps = [
        {
            "x": x[i * B_LOC : (i + 1) * B_LOC],
            "wm": wm,
            "wm0": wm0,
            "wd": wd,
        }
        for i in range(N_CORES)
    ]
    out = bass_utils.run_bass_kernel_spmd(
        nc, in_maps, core_ids=list(range(N_CORES)), trace=trace
    )
    res = np.concatenate([o["res"] for o in out.results], axis=0)
    ma = np.concatenate([o["ma"] for o in out.results], axis=0)
    return res, ma, out


def kernel(x: np.ndarray):
    res, ma, _ = _run(x, trace=False)
    return res, ma



# revision 2
# speedup vs baseline: 1.1162x; 1.1162x over previous
"""EMA-decomposition kernel for Trainium2 (8 NeuronCores, Bass/Tile).

Problem: x [32, 4096, 512] f32; EMA along time (alpha=0.3):
    s_0 = x_0, s_t = a*x_t + (1-a)*s_{t-1}
Returns (x - s, s).

Math: with a=0.3 the per-128-step block decay (0.7)^128 ~ 1.5e-20 is far
below fp32 resolution, so the scan carry beyond one 128-step block is
numerically zero.  Each 128-row output block is exactly (to fp32):
    s_blk[j] = M  @ x_blk[j] + D @ x_blk[j-1]      (j >= 1)
    s_blk[0] = M0 @ x_blk[0]
with constant 128x128 matrices
    M[t,k]  = a*(1-a)^(t-k)  for k<=t else 0
    M0      = M with row 0 replaced by (1-a)^t  (s_0 = x_0 boundary)
    D[t,k]  = a*(1-a)^(t+128-k)
so the scan becomes independent TensorE matmuls.  Matmuls run at
f32r (tf32) rate; absmax rel error ~1.7e-4 (gate 2e-2).

Sharding: batch 32 -> 4 sequences/core over 8 cores (time axis never
sharded).  Per-core traffic 32 MiB in + 64 MiB out = 96 MiB; HBM bound
~358 GB/s/NC -> ~281 us theoretical floor.  Measured pure-DMA floor for
this pattern ~298 us; this kernel ~324 us (repeat-slope, min estimator).

Schedule (per core), chosen by in-batch HW A/B:
  SP   x-in megatile DMAs (2 MiB) + res-out DMAs.  Input prefetch runs
       ONE SEQUENCE AHEAD of compute (x for seq b+2 is issued right
       after the res DMAs of seq b), so prefetch never starves behind
       data-dependent res waits.
  ACT  psum->sbuf eviction of ma (per block) + ma-out DMAs.
  DVE  per-block f32->f32r rounding copies into a small ring (the BIR
       verifier requires f32r matmul operands to come from a rounding
       op), emitted one block ahead; one whole-megatile res sub
       (res = x - mat) in place into the x tile.
  PE   2 matmuls per block (M @ xr_j, D @ xr_{j-1}), N=512, one PSUM
       bank each, 8 banks rotating.
  No gpsimd: a 3rd SWDGE queue measured ~15 us SLOWER than this
  2-queue HWDGE layout, and SWDGE DMAs break walrus codegen inside
  tc.For_i (the bench wrapper) anyway.  Bench variant == graded variant.
"""

import numpy as np

import concourse.bass as bass
import concourse.mybir as mybir
from concourse import bass_utils
from concourse.tile import TileContext

ALPHA = 0.3
B, L, C = 32, 4096, 512
N_CORES = 8
B_LOC = B // N_CORES          # 4 sequences per core
P = 128                       # partition dim == time-block size
N_BLK = L // P                # 32 blocks per sequence
MEGA = 8                      # blocks per megatile (DMA granularity: 2 MiB)
N_MEGA = N_BLK // MEGA        # 4 megatiles per sequence


def _build_weights():
    """lhsT layouts ([k, t] so that out = lhsT.T @ rhs)."""
    a = float(ALPHA)
    q = 1.0 - a
    k = np.arange(P, dtype=np.float64)[:, None]
    t = np.arange(P, dtype=np.float64)[None, :]
    e = t - k
    with np.errstate(under="ignore"):
        lhsT_m = np.where(e >= 0, a * q ** np.maximum(e, 0.0), 0.0)
        lhsT_m0 = lhsT_m.copy()
        lhsT_m0[0, :] = q ** t[0]
        lhsT_d = a * q ** (e + P)
    return (
        lhsT_m.astype(np.float32),
        lhsT_m0.astype(np.float32),
        lhsT_d.astype(np.float32),
    )


def _build_bass(repeat: int = 1) -> bass.Bass:
    """repeat>1 wraps the body in a For_i hardware loop — bench only."""
    nc = bass.Bass(trn_type="TRN2")
    f32 = mybir.dt.float32
    f32r = mybir.dt.float32r

    x_d = nc.dram_tensor("x", [B_LOC, L, C], f32, kind="ExternalInput")
    wm_d = nc.dram_tensor("wm", [P, P], f32, kind="ExternalInput")
    wm0_d = nc.dram_tensor("wm0", [P, P], f32, kind="ExternalInput")
    wd_d = nc.dram_tensor("wd", [P, P], f32, kind="ExternalInput")
    res_d = nc.dram_tensor("res", [B_LOC, L, C], f32, kind="ExternalOutput")
    ma_d = nc.dram_tensor("ma", [B_LOC, L, C], f32, kind="ExternalOutput")

    with TileContext(nc) as tc:
        with (
            tc.tile_pool(name="wpool", bufs=1) as wpool,
            tc.tile_pool(name="xpool", bufs=8) as xpool,
            tc.tile_pool(name="xrpool", bufs=6) as xrpool,
            tc.tile_pool(name="mapool", bufs=2) as mapool,
            tc.tile_pool(name="pspool", bufs=8, space="PSUM") as pspool,
        ):
            w = {}
            for name, dram in (("m", wm_d), ("m0", wm0_d), ("d", wd_d)):
                t = wpool.tile([P, P], f32, name=f"w32_{name}")
                nc.scalar.dma_start(out=t, in_=dram[:, :])
                wr = wpool.tile([P, P], f32r, name=f"wr_{name}")
                nc.vector.tensor_copy(out=wr, in_=t)
                w[name] = wr

            def emit_x(b, xtiles):
                xr_ = x_d[b].rearrange("(g j p) c -> g p j c", j=MEGA, p=P)
                tiles = []
                for g in range(N_MEGA):
                    xt = xpool.tile([P, MEGA, C], f32, name="xt")
                    nc.sync.dma_start(out=xt, in_=xr_[g])
                    tiles.append(xt)
                xtiles[b] = tiles

            def compute_seq(b, xtiles):
                mar = ma_d[b].rearrange("(g j p) c -> g p j c", j=MEGA, p=P)
                resr = res_d[b].rearrange("(g j p) c -> g p j c", j=MEGA, p=P)
                xts = xtiles[b]
                res_emits = []
                rounds = {}

                def ensure_round(k):
                    if k < N_BLK and k not in rounds:
                        g, j = divmod(k, MEGA)
                        xrb = xrpool.tile([P, C], f32r, name="xrb")
                        nc.vector.tensor_copy(out=xrb, in_=xts[g][:, j, :])
                        rounds[k] = xrb

                ensure_round(0)
                for g in range(N_MEGA):
                    xt = xts[g]
                    mat = mapool.tile([P, MEGA, C], f32, name="mat")
                    for j in range(MEGA):
                        k = g * MEGA + j
                        ensure_round(k + 1)
                        ps = pspool.tile([P, C], f32, name="ps")
                        cur = rounds[k]
                        if k == 0:
                            nc.tensor.matmul(
                                ps, w["m0"], cur, start=True, stop=True
                            )
                        else:
                            nc.tensor.matmul(
                                ps, w["m"], cur, start=True, stop=False
                            )
                            nc.tensor.matmul(
                                ps, w["d"], rounds[k - 1],
                                start=False, stop=True,
                            )
                            del rounds[k - 1]
                        nc.scalar.copy(out=mat[:, j, :], in_=ps)
                    # res = x - ma for the whole megatile, in place into
                    # the x tile (PE only ever reads the rounded ring).
                    nc.vector.tensor_sub(out=xt, in0=xt, in1=mat)
                    nc.scalar.dma_start(out=mar[g], in_=mat)
                    res_emits.append((resr, g, xt))
                return res_emits

            def emit_res_dmas(res_emits):
                for resr, g, xt in res_emits:
                    nc.sync.dma_start(out=resr[g], in_=xt)

            def body():
                xtiles = {}
                emit_x(0, xtiles)
                emit_x(1, xtiles)
                for b in range(B_LOC):
                    res_emits = compute_seq(b, xtiles)
                    emit_res_dmas(res_emits)
                    if b + 2 < B_LOC:
                        emit_x(b + 2, xtiles)

            if repeat > 1:
                with tc.For_i(0, repeat, 1):
                    body()
            else:
                body()
    return nc


def _split_multi_waits(nc: bass.Bass) -> None:
    """Walrus codegen in this container allows only ONE semaphore wait per
    instruction ("Too many sync wait commands").  Tile's sem assigner emits
    several.  Split: hoist all but one wait onto same-engine NoOps placed
    immediately before the instruction (engines execute their stream in
    order, so this is semantically identical)."""
    n_nops = 0
    for fn in nc.m.functions:
        for blk in fn.blocks:
            out = []
            for inst in blk.instructions:
                si = inst.sync_info
                if si is not None and si.on_wait and len(si.on_wait) > 1:
                    waits = list(si.on_wait)
                    for w_ in waits[:-1]:
                        nop = mybir.InstNoOp(
                            name=f"{inst.name}-wsplit{n_nops}",
                            engine=inst.engine,
                            ins=[],
                            outs=[],
                        )
                        nop.sync_info = mybir.SyncInfo(on_wait=[w_], on_update=[])
                        out.append(nop)
                        n_nops += 1
                    si.on_wait = [waits[-1]]
                out.append(inst)
            blk.instructions = out


def _run(x: np.ndarray, trace: bool = False):
    x = np.ascontiguousarray(np.asarray(x, dtype=np.float32))
    assert x.shape == (B, L, C), x.shape
    wm, wm0, wd = _build_weights()
    nc = _build_bass()
    _split_multi_waits(nc)
    in_maps = [
        {
            "x": x[i * B_LOC : (i + 1) * B_LOC],
            "wm": wm,
            "wm0": wm0,
            "wd": wd,
        }
        for i in range(N_CORES)
    ]
    out = bass_utils.run_bass_kernel_spmd(
        nc, in_maps, core_ids=list(range(N_CORES)), trace=trace
    )
    res = np.concatenate([o["res"] for o in out.results], axis=0)
    ma = np.concatenate([o["ma"] for o in out.results], axis=0)
    return res, ma, out


def kernel(x: np.ndarray):
    res, ma, _ = _run(x, trace=False)
    return res, ma
